# revision 2
# baseline (speedup 1.0000x reference)
import functools

import jax
import jax.numpy as jnp
import numpy as np

# nn_CapLayer: grouped 1x1 conv + 3-iter dynamic routing (capsule layer).
#
# Data-parallel over batch: 256 batch elements sharded 32-per-core across
# 8 NeuronCores; conv weight w and bias b_conv replicated. Routing is
# batch-local so there is no cross-device communication.
#
# The routing is computed in FACTORED form: the (bs, 10, 16, 1152) `pred`
# tensor (189 MB fp32 over the full batch) is never materialized. Every
# contraction against pred is pushed through its low-rank structure
# pred = W.xg + bias, so the kernel is bounded by reading x once plus
# small per-iteration tensors.
#
# Optimizations over the straightforward factored form:
#  * bias folded into the contractions: x is augmented with a constant-1
#    input-capsule row and W with the bias column, so the h/cs bias terms
#    ride the same einsums instead of separate kernels.
#  * heavy contractions run in bf16 (PE runs bf16 at 4x the fp32 rate on
#    trn2) with fp32 accumulation; squash/softmax stay fp32.
#  * softmax skips the max-subtraction (logits are O(1) here: |L| < ~4,
#    exp is safe in fp32) and defers the 1/sum(e) normalization past the
#    linear contractions: s_r = (e . pred)/Z instead of (e/Z) . pred,
#    replacing a (b,j,1152)-sized divide with a (b,j,16)-sized one.

NUM_SHARED = 32
IN_DIM = 8
NUM_OUT = 10
OUT_DIM = 16
ROUTE_NUM = 3
EPS = 1e-20

N_CORES = 8
BS = 256
H = 6
P = H * H  # 36 spatial positions
I = NUM_SHARED * P  # 1152 input capsules

BF = jnp.bfloat16
F32 = jnp.float32


def _squash(s):
    # s: (bs, J, D) fp32
    n2 = jnp.sum(s * s, axis=2, keepdims=True)
    n = jnp.sqrt(n2)
    return s * (n2 / (1.0 + n2) / (n + EPS))


def _caps_shard(x, w, b_conv):
    # x: (bs_l, NUM_SHARED*IN_DIM, h, h) on one core
    bs = x.shape[0]
    xg = x.reshape(bs, NUM_SHARED, IN_DIM, P)               # (b, s, i', p)
    # Augment i' with a constant-1 row so the conv bias contributes via
    # the same contractions (pred = Wa . xga with Wa[..., -1] = bias).
    ones = jnp.ones((bs, NUM_SHARED, 1, P), x.dtype)
    xga = jnp.concatenate([xg, ones], axis=2)               # (b, s, 9, p)
    xgb = xga.astype(BF)

    Wr = w.reshape(NUM_SHARED, NUM_OUT, OUT_DIM, IN_DIM)    # (s, j, d, i')
    Br = b_conv.reshape(NUM_SHARED, NUM_OUT, OUT_DIM)       # (s, j, d)
    Wa = jnp.concatenate([Wr, Br[..., None]], axis=3)       # (s, j, d, 9)
    Wab = Wa.astype(BF)

    # r = 0: c is uniform (softmax of zeros) -> s0 = mean_i pred
    xs0 = jnp.sum(xga, axis=3)                              # (b, s, 9)
    s0 = jnp.einsum('bsi,sjdi->bjd', xs0, Wa) * (1.0 / I)
    v = _squash(s0)

    L = None  # routing logits, (b, j, s, p) fp32; None means all-zero
    for r in range(1, ROUTE_NUM):
        # b-update with v from iteration r-1:
        #   dL[b,j,s,p] = sum_i'' g[b,j,s,i''] * xga[b,s,i'',p]
        # (the i''=8 slot carries the bias term h automatically)
        g = jnp.einsum('bjd,sjdi->bjsi', v.astype(BF), Wab,
                       preferred_element_type=F32).astype(BF)
        dL = jnp.einsum('bjsi,bsip->bjsp', g, xgb,
                        preferred_element_type=F32)         # fp32
        L = dL if L is None else L + dL

        # softmax over i=(s,p) without max-subtraction, normalization
        # deferred to s_r
        e = jnp.exp(L)
        Z = jnp.sum(e, axis=(2, 3))                         # (b, j)
        ya = jnp.einsum('bjsp,bsip->bjsi', e.astype(BF), xgb,
                        preferred_element_type=F32)         # (b, j, s, 9)
        s_r = jnp.einsum('bjsi,sjdi->bjd', ya.astype(BF), Wab,
                         preferred_element_type=F32)
        s_r = s_r / Z[:, :, None]
        v = _squash(s_r)
    return v


@functools.cache
def _pmapped(n_cores: int):
    return jax.pmap(_caps_shard, axis_name='cores', devices=jax.devices()[:n_cores])


@functools.cache
def _pmapped_loop(n_cores: int, n_iter: int):
    # Timing helper: runs the shard computation n_iter times back-to-back
    # on-device inside one dispatch, chaining a data dependency through
    # the input so XLA cannot hoist or CSE the iterations. Device time of
    # one iteration = (t_loop - dispatch_floor) / n_iter.
    def run(x, w, b_conv):
        def body(carry, _):
            v = _caps_shard(x * (1.0 + carry), w, b_conv)
            return jnp.max(jnp.abs(v)) * 1e-30, None
        c, _ = jax.lax.scan(body, jnp.float32(0.0), None, length=n_iter)
        return c
    return jax.pmap(run, axis_name='cores', devices=jax.devices()[:n_cores])


def kernel(x: np.ndarray, w: np.ndarray, b_conv: np.ndarray) -> np.ndarray:
    bs = x.shape[0]
    n_cores = N_CORES
    n_dev = len(jax.devices())
    while n_cores > 1 and (n_cores > n_dev or bs % n_cores != 0):
        n_cores //= 2
    shard = bs // n_cores
    xs = np.ascontiguousarray(x.reshape(n_cores, shard, *x.shape[1:]))
    ws = np.ascontiguousarray(np.broadcast_to(w, (n_cores,) + w.shape))
    bs_ = np.ascontiguousarray(np.broadcast_to(b_conv, (n_cores,) + b_conv.shape))
    v = _pmapped(n_cores)(xs, ws, bs_)
    v = np.asarray(v)
    return v.reshape(bs, NUM_OUT, OUT_DIM)
